# revision 5
# baseline (speedup 1.0000x reference)
"""Trainium2 Bass kernel for BGEM3 sparse-embedding head (segment_reduce).

Computes, for inputs hidden_state [B,S,H], input_ids [B,S], W_sparse [1,H],
b_sparse [1]:
    tw = relu(hidden_state @ W_sparse[0] + b_sparse[0])          # [B,S]
    out = zeros([B,V]); out[b, ids[b,s]] = max(out[...], tw[b,s])
    out[:, 0:4] = 0
Sharding: data-parallel over batch across 8 NeuronCores (4 rows per core).

Per core, per batch row:
  1. matvec: 8 x fused mult+add-reduce (DVE) against a broadcast W tile
  2. relu + special-token mask
  3. row-global duplicate resolution: every token's weight is replaced by the
     SUM of weights over all tokens in the row sharing its id (via a broadcast
     id/weight row + is_equal mask + add-reduce). This matches the oracle:
     jax's .at[].max() lowered on this stack sums duplicate indices (and all
     weights are >= 0 post-relu, so non-duplicates equal the plain value).
     After this, colliding scatter writes all carry bit-identical values, so
     plain (bypass) DMA scatter is order-independent and exact.
  4. one indirect DMA scatters the 1024 (id, weight) pairs into the row's
     [Vpad,1] table in DRAM.
"""

import numpy as np

B, S, H, V = 32, 1024, 1024, 250002
N_CORES = 8
B_LOC = B // N_CORES          # 4 batch rows per core
VPAD = 128 * 1954             # 250112 >= V, divisible by 128
N_STILE = S // 128            # 8 s-tiles per row

# Zero-init the output tables with an explicit DMA. The PJRT path donates
# zero buffers as outputs, which would make this redundant, but an explicit
# init keeps correctness independent of that mechanism.
ZERO_INIT = True

_compiled = {}


def _build(b_val: float):
    import concourse.bass as bass
    import concourse.tile as tile
    from concourse import bacc, mybir

    f32 = mybir.dt.float32
    i32 = mybir.dt.int32
    Alu = mybir.AluOpType

    nc = bacc.Bacc("TRN2", target_bir_lowering=False, debug=False)

    hs = nc.dram_tensor("hs", [B_LOC, S, H], f32, kind="ExternalInput")
    ids = nc.dram_tensor("ids", [B_LOC, S], i32, kind="ExternalInput")
    w = nc.dram_tensor("w", [1, H], f32, kind="ExternalInput")
    tables = [
        nc.dram_tensor(f"t{r}", [VPAD, 1], f32, kind="ExternalOutput")
        for r in range(B_LOC)
    ]

    with tile.TileContext(nc) as tc:
        with (
            tc.tile_pool(name="wb", bufs=1) as wb_pool,
            tc.tile_pool(name="zeros", bufs=1) as z_pool,
            tc.tile_pool(name="h", bufs=4) as h_pool,
            tc.tile_pool(name="bcast", bufs=2) as bc_pool,
            tc.tile_pool(name="scratch", bufs=2) as sc_pool,
            tc.tile_pool(name="small", bufs=2) as sm_pool,
            tc.tile_pool(name="dram", bufs=1, space="DRAM") as dram_pool,
        ):
            # one-time: W broadcast to all 128 partitions
            w_bc = wb_pool.tile([128, H], f32)
            nc.sync.dma_start(w_bc[:], w[0:1, :].to_broadcast([128, H]))

            if ZERO_INIT:
                zt = z_pool.tile([128, VPAD // 128], f32)
                nc.vector.memset(zt[:], 0.0)
                for r in range(B_LOC):
                    dst = tables[r][:].rearrange("(p x) 1 -> p x", p=128)
                    nc.sync.dma_start(dst, zt[:])

            for r in range(B_LOC):
                # ---- matvec: tw[p, j] = b + sum_h hs[r, 128j+p, :] * W ----
                twraw = sm_pool.tile([128, N_STILE], f32, tag="twraw")
                for j in range(N_STILE):
                    ht = h_pool.tile([128, H], f32, tag="h")
                    nc.sync.dma_start(ht[:], hs[r, 128 * j : 128 * (j + 1), :])
                    prod = sc_pool.tile([128, H], f32, tag="prod")
                    # (ht * 1.0) * w_bc, accum_out = row-sum -> dot product.
                    # (tensor_tensor_reduce crashes the HW runtime; STT+accum
                    # is the working fused multiply-reduce.)
                    nc.vector.scalar_tensor_tensor(
                        out=prod[:],
                        in0=ht[:],
                        scalar=1.0,
                        in1=w_bc[:],
                        op0=Alu.mult,
                        op1=Alu.mult,
                        accum_out=twraw[:, j : j + 1],
                    )

                # ---- relu(x + b) + mask ids<4 (column layout: s=128j+p) ----
                twrelu = sm_pool.tile([128, N_STILE], f32, tag="twrelu")
                nc.scalar.activation(
                    twrelu[:],
                    twraw[:],
                    mybir.ActivationFunctionType.Relu,
                    bias=float(b_val),
                )
                idc_i = sm_pool.tile([128, N_STILE], i32, tag="idc_i")
                nc.sync.dma_start(
                    idc_i[:], ids[r, :].rearrange("(j p) -> p j", p=128)
                )
                idc_f = sm_pool.tile([128, N_STILE], f32, tag="idc_f")
                nc.vector.tensor_copy(idc_f[:], idc_i[:])
                twm = sm_pool.tile([128, N_STILE], f32, tag="twm")
                nc.vector.scalar_tensor_tensor(
                    out=twm[:],
                    in0=idc_f[:],
                    scalar=4.0,
                    in1=twrelu[:],
                    op0=Alu.is_ge,
                    op1=Alu.mult,
                )

                # ---- bounce masked weights to DRAM in s-order ----
                scr = dram_pool.tile([S], f32, tag="scr")
                nc.sync.dma_start(
                    scr[:].rearrange("(j p) -> p j", p=128), twm[:]
                )

                # ---- broadcast rows: twT[p,q]=tw[q], idT[p,q]=id[q] ----
                twT = bc_pool.tile([128, S], f32, tag="twT")
                nc.sync.dma_start(twT[:], scr[:][None, :].to_broadcast([128, S]))
                idT_i = bc_pool.tile([128, S], i32, tag="idT_i")
                nc.sync.dma_start(
                    idT_i[:], ids[r, :][None, :].to_broadcast([128, S])
                )
                idT_f = bc_pool.tile([128, S], f32, tag="idT_f")
                nc.vector.tensor_copy(idT_f[:], idT_i[:])

                # ---- natural-layout ids ((p,j) <-> s=8p+j) for scatter ----
                idn_i = sm_pool.tile([128, N_STILE], i32, tag="idn_i")
                nc.sync.dma_start(
                    idn_i[:], ids[r, :].rearrange("(p j) -> p j", j=N_STILE)
                )
                idn_f = sm_pool.tile([128, N_STILE], f32, tag="idn_f")
                nc.vector.tensor_copy(idn_f[:], idn_i[:])

                # ---- row-global dup-max: twd[p,j] = max_q eq * tw[q] ----
                twd = sm_pool.tile([128, N_STILE], f32, tag="twd")
                for j in range(N_STILE):
                    masked = sc_pool.tile([128, S], f32, tag="masked")
                    nc.vector.scalar_tensor_tensor(
                        out=masked[:],
                        in0=idT_f[:],
                        scalar=idn_f[:, j : j + 1],
                        in1=twT[:],
                        op0=Alu.is_equal,
                        op1=Alu.mult,
                    )
                    nc.vector.reduce_sum(
                        out=twd[:, j : j + 1],
                        in_=masked[:],
                        axis=mybir.AxisListType.X,
                    )

                # ---- scatter (bypass; duplicates carry identical values) ----
                # One offset per SBUF partition per indirect DMA -> 8 column
                # scatters of 128 single-element writes each.
                for j in range(N_STILE):
                    nc.gpsimd.indirect_dma_start(
                        out=tables[r][:],
                        out_offset=bass.IndirectOffsetOnAxis(
                            ap=idn_i[:, j : j + 1], axis=0
                        ),
                        in_=twd[:, j : j + 1],
                        in_offset=None,
                    )

    nc.compile()
    return nc


def _get_nc(b_val: float):
    key = float(b_val)
    if key not in _compiled:
        _compiled[key] = _build(key)
    return _compiled[key]


def kernel(hidden_state, input_ids, W_sparse, b_sparse):
    from concourse.bass_utils import run_bass_kernel_spmd

    hidden_state = np.ascontiguousarray(np.asarray(hidden_state, dtype=np.float32))
    input_ids = np.ascontiguousarray(np.asarray(input_ids, dtype=np.int32))
    W_sparse = np.ascontiguousarray(np.asarray(W_sparse, dtype=np.float32))
    b_val = float(np.asarray(b_sparse).reshape(-1)[0])

    nc = _get_nc(b_val)

    in_maps = []
    for c in range(N_CORES):
        sl = slice(c * B_LOC, (c + 1) * B_LOC)
        in_maps.append(
            {"hs": hidden_state[sl], "ids": input_ids[sl], "w": W_sparse}
        )

    res = run_bass_kernel_spmd(nc, in_maps, list(range(N_CORES)))

    out = np.empty((B, V), dtype=np.float32)
    for c in range(N_CORES):
        for r in range(B_LOC):
            out[c * B_LOC + r] = res.results[c][f"t{r}"][:V, 0]
    return out


# revision 6
# speedup vs baseline: 1.5176x; 1.5176x over previous
"""Trainium2 Bass kernel for BGEM3 sparse-embedding head (segment_reduce).

Computes, for inputs hidden_state [B,S,H], input_ids [B,S], W_sparse [1,H],
b_sparse [1]:
    tw = relu(hidden_state @ W_sparse[0] + b_sparse[0])          # [B,S]
    out = zeros([B,V]); out.at[b, ids].max(tw)  (jax scatter-max, which on
    this stack sums duplicate indices); out[:, 0:4] = 0
Sharding: data-parallel over batch across 8 NeuronCores (4 rows per core).

Per core, per batch row (8 column-tiles of 128 tokens each):
  1. matvec: fused mult + add-reduce (DVE scalar_tensor_tensor + accum)
     against a broadcast W tile; relu(x+b) on ACT.
  2. per-column duplicate resolution:
       eq[p,q]  = (id[p] == id[q])            (PE transpose + DVE STT)
       cnt[p]   = sum_q eq * tril             (DVE STT + accum: # of earlier
                                               duplicates -> carrier iff 0)
       gsum[p]  = sum_q eq[p,q] * tw[q]       (PE matmul; eq is symmetric)
     Non-carrier tokens get their id remapped out of bounds so the scatter
     drops them: no instruction ever contains duplicate offsets (the DMA's
     within-instruction read-modify-write races).
  3. 8 indirect cce-add scatters per row, sequenced by Tile (same-tensor
     WAW), so cross-column duplicates accumulate exactly.
"""

import numpy as np

B, S, H, V = 32, 1024, 1024, 250002
N_CORES = 8
B_LOC = B // N_CORES          # 4 batch rows per core
VPAD = 128 * 1954             # 250112 >= V, divisible by 128
N_STILE = S // 128            # 8 column-tiles per row
BIG = 524288.0                # OOB offset for dropped (non-carrier) tokens

ZERO_INIT = True

_compiled = {}


def _build(b_val: float):
    import concourse.bass as bass
    import concourse.tile as tile
    from concourse import bacc, mybir
    from concourse.masks import make_identity

    f32 = mybir.dt.float32
    i32 = mybir.dt.int32
    Alu = mybir.AluOpType

    nc = bacc.Bacc("TRN2", target_bir_lowering=False, debug=False)

    hs = nc.dram_tensor("hs", [B_LOC, S, H], f32, kind="ExternalInput")
    ids = nc.dram_tensor("ids", [B_LOC, S], i32, kind="ExternalInput")
    w = nc.dram_tensor("w", [1, H], f32, kind="ExternalInput")
    tables = [
        nc.dram_tensor(f"t{r}", [VPAD, 1], f32, kind="ExternalOutput")
        for r in range(B_LOC)
    ]

    with tile.TileContext(nc) as tc:
        with (
            tc.tile_pool(name="const", bufs=1) as const_pool,
            tc.tile_pool(name="h", bufs=4) as h_pool,
            tc.tile_pool(name="sc", bufs=3) as sc_pool,
            tc.tile_pool(name="sm", bufs=2) as sm_pool,
            tc.tile_pool(name="ps", bufs=4, space="PSUM") as ps_pool,
        ):
            # ---- one-time constants ----
            w_bc = const_pool.tile([128, H], f32)
            nc.sync.dma_start(w_bc[:], w[0:1, :].to_broadcast([128, H]))

            ident = const_pool.tile([128, 128], f32)
            make_identity(nc, ident[:])

            ones = const_pool.tile([128, 128], f32)
            nc.vector.memset(ones[:], 1.0)
            tril = const_pool.tile([128, 128], f32)
            # tril[p, q] = 1.0 where p - q > 0 else 0.0
            nc.gpsimd.affine_select(
                out=tril[:],
                in_=ones[:],
                pattern=[[-1, 128]],
                compare_op=Alu.is_gt,
                fill=0.0,
                base=0,
                channel_multiplier=1,
            )

            if ZERO_INIT:
                zt = const_pool.tile([128, VPAD // 128], f32)
                nc.vector.memset(zt[:], 0.0)
                for r in range(B_LOC):
                    dst = tables[r][:].rearrange("(p x) 1 -> p x", p=128)
                    nc.sync.dma_start(dst, zt[:])

            for r in range(B_LOC):
                # ---- matvec: twraw[p, j] = sum_h hs[r, 128j+p, :] * W ----
                twraw = sm_pool.tile([128, N_STILE], f32, tag="twraw")
                for j in range(N_STILE):
                    ht = h_pool.tile([128, H], f32, tag="h")
                    nc.sync.dma_start(ht[:], hs[r, 128 * j : 128 * (j + 1), :])
                    prod = sc_pool.tile([128, H], f32, tag="prod")
                    nc.vector.scalar_tensor_tensor(
                        out=prod[:],
                        in0=ht[:],
                        scalar=1.0,
                        in1=w_bc[:],
                        op0=Alu.mult,
                        op1=Alu.mult,
                        accum_out=twraw[:, j : j + 1],
                    )

                # ---- relu(x + b); mask special tokens (ids < 4) to 0 ----
                twrelu = sm_pool.tile([128, N_STILE], f32, tag="twrelu")
                nc.scalar.activation(
                    twrelu[:],
                    twraw[:],
                    mybir.ActivationFunctionType.Relu,
                    bias=float(b_val),
                )
                idc_i = sm_pool.tile([128, N_STILE], i32, tag="idc_i")
                nc.sync.dma_start(
                    idc_i[:], ids[r, :].rearrange("(j p) -> p j", p=128)
                )
                idc_f = sm_pool.tile([128, N_STILE], f32, tag="idc_f")
                nc.vector.tensor_copy(idc_f[:], idc_i[:])
                twm = sm_pool.tile([128, N_STILE], f32, tag="twm")
                nc.vector.scalar_tensor_tensor(
                    out=twm[:],
                    in0=idc_f[:],
                    scalar=4.0,
                    in1=twrelu[:],
                    op0=Alu.is_ge,
                    op1=Alu.mult,
                )

                # ---- per-column dedup ----
                cnt8 = sm_pool.tile([128, N_STILE], f32, tag="cnt8")
                gsum_ps = ps_pool.tile([128, N_STILE], f32, tag="gsum")
                for j in range(N_STILE):
                    idT = ps_pool.tile([128, 128], f32, tag="idT")
                    nc.tensor.transpose(
                        out=idT[:],
                        in_=idc_f[:, j : j + 1].to_broadcast([128, 128]),
                        identity=ident[:],
                    )
                    eq = sc_pool.tile([128, 128], f32, tag="eq")
                    nc.vector.scalar_tensor_tensor(
                        out=eq[:],
                        in0=idT[:],
                        scalar=idc_f[:, j : j + 1],
                        in1=ones[:],
                        op0=Alu.is_equal,
                        op1=Alu.bypass,
                    )
                    cntscr = sc_pool.tile([128, 128], f32, tag="cntscr")
                    nc.vector.scalar_tensor_tensor(
                        out=cntscr[:],
                        in0=eq[:],
                        scalar=1.0,
                        in1=tril[:],
                        op0=Alu.mult,
                        op1=Alu.mult,
                        accum_out=cnt8[:, j : j + 1],
                    )
                    # gsum[p] = sum_q eq[q, p] * twm[q]  (eq symmetric)
                    nc.tensor.matmul(
                        out=gsum_ps[:, j : j + 1],
                        lhsT=eq[:],
                        rhs=twm[:, j : j + 1],
                        start=True,
                        stop=True,
                    )

                gsum = sm_pool.tile([128, N_STILE], f32, tag="gsumsb")
                nc.vector.tensor_copy(gsum[:], gsum_ps[:])

                # ---- remap non-carriers out of bounds, convert to int ----
                nb = sm_pool.tile([128, N_STILE], f32, tag="nb")
                nc.vector.tensor_scalar(
                    out=nb[:],
                    in0=cnt8[:],
                    scalar1=0.0,
                    op0=Alu.not_equal,
                    scalar2=BIG,
                    op1=Alu.mult,
                )
                idx_f = sm_pool.tile([128, N_STILE], f32, tag="idx_f")
                nc.vector.tensor_add(idx_f[:], idc_f[:], nb[:])
                idx_i = sm_pool.tile([128, N_STILE], i32, tag="idx_i")
                nc.vector.tensor_copy(idx_i[:], idx_f[:])

                # ---- sequenced cce-add scatters (128 offsets each) ----
                for j in range(N_STILE):
                    nc.gpsimd.indirect_dma_start(
                        out=tables[r][:],
                        out_offset=bass.IndirectOffsetOnAxis(
                            ap=idx_i[:, j : j + 1], axis=0
                        ),
                        in_=gsum[:, j : j + 1],
                        in_offset=None,
                        compute_op=Alu.add,
                        bounds_check=V - 1,
                        oob_is_err=False,
                    )

    nc.compile()
    return nc


def _get_nc(b_val: float):
    key = float(b_val)
    if key not in _compiled:
        _compiled[key] = _build(key)
    return _compiled[key]


def kernel(hidden_state, input_ids, W_sparse, b_sparse):
    from concourse.bass_utils import run_bass_kernel_spmd

    hidden_state = np.ascontiguousarray(np.asarray(hidden_state, dtype=np.float32))
    input_ids = np.ascontiguousarray(np.asarray(input_ids, dtype=np.int32))
    W_sparse = np.ascontiguousarray(np.asarray(W_sparse, dtype=np.float32))
    b_val = float(np.asarray(b_sparse).reshape(-1)[0])

    nc = _get_nc(b_val)

    in_maps = []
    for c in range(N_CORES):
        sl = slice(c * B_LOC, (c + 1) * B_LOC)
        in_maps.append(
            {"hs": hidden_state[sl], "ids": input_ids[sl], "w": W_sparse}
        )

    res = run_bass_kernel_spmd(nc, in_maps, list(range(N_CORES)))

    out = np.empty((B, V), dtype=np.float32)
    for c in range(N_CORES):
        for r in range(B_LOC):
            out[c * B_LOC + r] = res.results[c][f"t{r}"][:V, 0]
    return out
